# revision 7
# baseline (speedup 1.0000x reference)
"""Trainium2 Bass kernel for ActorCriticRNN (GRU scan + policy/value MLPs).

Sharding: data-parallel over batch B=256 -> 8 cores x 32. The GRU scan runs
in "transposed" layout (features/gates on SBUF partitions, batch on the free
dim) so per-gate biases fold into matmuls/ACT bias slots and no transposes
appear on the 1024-step critical path. Everything on-chip is bf16 with fp32
PSUM accumulation. The policy/value MLPs consume the hidden-state blocks in
the same transposed layout and are interleaved into the scan's idle PE/ACT
slots.
"""

import numpy as np
import ml_dtypes

B, T, IN, R = 256, 1024, 64, 256
NCORES = 8
BL = B // NCORES          # 32 batch per core
G4 = 4                    # timesteps per x-projection group (psum-bank sized)
NBLK = 16                 # timesteps per MLP block (512 rows)
OBS_BLK = 128             # timesteps per obs/mask DMA block

bf16 = ml_dtypes.bfloat16

_NC_CACHE = {}


def _patch_tile_drain():
    """walrus on this stack rejects >1 semaphore wait on a CTRL instruction;
    spread the Tile tail-drain's waits over a chain of SP nops."""
    import concourse.tile as tile_mod
    from concourse.tile import ScopedClock
    import concourse.mybir as mybir

    if getattr(tile_mod.TileContext, "_drain_patched", False):
        return

    def _patched(self, tick_clock, wait_clock):
        nc = self.nc
        probe = nc.sync.nop()
        wait_clock.add_sem_waits(
            probe.ins, ScopedClock({None: tick_clock.global_clock})
        )
        waits = list(probe.ins.sync_info.on_wait) if probe.ins.sync_info else []
        if len(waits) > 1:
            probe.ins.sync_info.on_wait = waits[:1]
            for w in waits[1:]:
                n = nc.sync.nop()
                if n.ins.sync_info is None:
                    n.ins.sync_info = mybir.SyncInfo(on_wait=[w], on_update=[])
                else:
                    n.ins.sync_info.on_wait = [w]
        nc.sync.drain()
        nc.all_engine_barrier()
        assert self.sems is not None
        popped = nc._tile_sem_poison_stack.pop()
        assert popped is self._sem_poison
        nc.clear_and_free_semaphores(list(self.sems.allocated().values()))
        nc.all_engine_barrier()

    tile_mod.TileContext._drain_and_barrier = _patched
    tile_mod.TileContext._drain_patched = True



def _split_excess_waits(nc):
    """walrus's per-struct setupSyncWait limits: ~1 wait for CTRL (NoOp/Drain),
    ~2 for compute/DMA structs. Move excess semaphore waits onto same-engine
    NoOps inserted just before the offending instruction (waiting earlier on
    the same engine stream is semantically identical)."""
    import concourse.mybir as mybir

    k = 0
    for f in nc.m.functions:
        for bb in f.blocks:
            insts = list(bb.instructions)
            out = []
            for inst in insts:
                si = inst.sync_info
                lim = 1
                if si and si.on_wait and len(si.on_wait) > lim:
                    waits = list(si.on_wait)
                    excess, keep = waits[:-lim], waits[-lim:]
                    for w in excess:
                        n = mybir.InstNoOp(name=f"wsplit_{k}", ins=[], outs=[])
                        k += 1
                        n.engine = inst.engine
                        n.sync_info = mybir.SyncInfo(on_wait=[w], on_update=[])
                        out.append(n)
                    si.on_wait = keep
                out.append(inst)
            if len(out) != len(insts):
                bb.instructions = out
    return k


def build_module(t_total=T):
    """Build the per-core Bass module (same program on all 8 cores)."""
    _patch_tile_drain()
    import concourse.bass as bass
    import concourse.tile as tile
    from concourse import mybir
    from contextlib import ExitStack

    f32 = mybir.dt.float32
    b16 = mybir.dt.bfloat16
    SIG = mybir.ActivationFunctionType.Sigmoid
    TANH = mybir.ActivationFunctionType.Tanh
    RELU = mybir.ActivationFunctionType.Relu

    TB = t_total * BL
    n_grp = t_total // G4
    n_blk = t_total // NBLK
    obs_blk = min(OBS_BLK, t_total)
    n_obsblk = t_total // obs_blk

    nc = bass.Bass()

    obsT_d = nc.declare_dram_parameter("obsT", [IN + 1, TB], b16, isOutput=False)
    mask_d = nc.declare_dram_parameter("maskrep", [128, TB], b16, isOutput=False)
    hminit_d = nc.declare_dram_parameter("hminit", [128, 2 * BL], b16, isOutput=False)
    whh_d = nc.declare_dram_parameter("whh", [128, 2 * 6 * 128], b16, isOutput=False)
    wih_d = nc.declare_dram_parameter("wih", [IN + 1, 6 * 128], b16, isOutput=False)
    bn_d = nc.declare_dram_parameter("bn", [1, 256], b16, isOutput=False)
    wp1_d = nc.declare_dram_parameter("wp1", [128, 512], b16, isOutput=False)
    wp2_d = nc.declare_dram_parameter("wp2", [128, 512], b16, isOutput=False)
    wv1_d = nc.declare_dram_parameter("wv1", [128, 512], b16, isOutput=False)
    wv2_d = nc.declare_dram_parameter("wv2", [128, 512], b16, isOutput=False)
    w3_d = nc.declare_dram_parameter("w3", [128, 2 * 17], b16, isOutput=False)
    bp1_d = nc.declare_dram_parameter("bp1", [128, 2], f32, isOutput=False)
    bp2_d = nc.declare_dram_parameter("bp2", [128, 2], f32, isOutput=False)
    bv1_d = nc.declare_dram_parameter("bv1", [128, 2], f32, isOutput=False)
    bv2_d = nc.declare_dram_parameter("bv2", [128, 2], f32, isOutput=False)
    b3_d = nc.declare_dram_parameter("b3", [33, 1], f32, isOutput=False)

    import os
    DBG = bool(os.environ.get("K_DEBUG"))
    if DBG:
        dbg_an_d = nc.declare_dram_parameter("dbg_an", [128, 2 * BL], f32, isOutput=True)
        dbg_rz_d = nc.declare_dram_parameter("dbg_rz", [128, 4 * BL], f32, isOutput=True)
    pol_d = nc.declare_dram_parameter("policy_out", [16, TB], f32, isOutput=True)
    val_d = nc.declare_dram_parameter("value_out", [1, TB], f32, isOutput=True)
    hf_d = nc.declare_dram_parameter("hfinal_out", [128, 2 * BL], f32, isOutput=True)

    with tile.TileContext(nc) as tc:
        with ExitStack() as ctx:
            P = lambda name, bufs, **kw: ctx.enter_context(
                tc.tile_pool(name=name, bufs=bufs, **kw)
            )
            consts = P("consts", 1)
            obs_pool = P("obsp", 2)
            mask_pool = P("maskp", 2)
            prz_pool = P("prz", 2, space="PSUM")
            pnx_pool = P("pnx", 2, space="PSUM")
            pmlp_pool = P("pmlp", 2, space="PSUM")
            pout_pool = P("pout", 1, space="PSUM")
            an_pool = P("anp", 2)
            rz_pool = P("rzp", 2)
            n_pool = P("np", 2)
            tmp_pool = P("tmpp", 2)
            hm_pool = P("hmp", 3)
            h1_pool = P("h1p", 2)
            x1_pool = P("x1p", 2)
            x2_pool = P("x2p", 2)
            outsb_pool = P("outsb", 2)

            # --- persistent constants ---
            whh_t = consts.tile([128, 2 * 6 * 128], b16, tag="whh")
            nc.sync.dma_start(whh_t[:], whh_d[:])
            wih_t = consts.tile([IN + 1, 6 * 128], b16, tag="wih")
            nc.sync.dma_start(wih_t[:], wih_d[:])
            bn_t = consts.tile([1, 256], b16, tag="bn")
            nc.sync.dma_start(bn_t[:], bn_d[:])
            ones_t = consts.tile([1, BL], b16, tag="ones")
            nc.vector.memset(ones_t[:], 1.0)
            wp1_t = consts.tile([128, 512], b16, tag="wp1")
            nc.sync.dma_start(wp1_t[:], wp1_d[:])
            wp2_t = consts.tile([128, 512], b16, tag="wp2")
            nc.sync.dma_start(wp2_t[:], wp2_d[:])
            wv1_t = consts.tile([128, 512], b16, tag="wv1")
            nc.sync.dma_start(wv1_t[:], wv1_d[:])
            wv2_t = consts.tile([128, 512], b16, tag="wv2")
            nc.sync.dma_start(wv2_t[:], wv2_d[:])
            w3_t = consts.tile([128, 2 * 17], b16, tag="w3")
            nc.sync.dma_start(w3_t[:], w3_d[:])
            bp1_t = consts.tile([128, 2], f32, tag="bp1")
            nc.sync.dma_start(bp1_t[:], bp1_d[:])
            bp2_t = consts.tile([128, 2], f32, tag="bp2")
            nc.sync.dma_start(bp2_t[:], bp2_d[:])
            bv1_t = consts.tile([128, 2], f32, tag="bv1")
            nc.sync.dma_start(bv1_t[:], bv1_d[:])
            bv2_t = consts.tile([128, 2], f32, tag="bv2")
            nc.sync.dma_start(bv2_t[:], bv2_d[:])
            b3_t = consts.tile([33, 1], f32, tag="b3")
            nc.sync.dma_start(b3_t[:], b3_d[:])

            # --- obs/mask block DMA ---
            obs_blocks = {}

            def emit_obsblock(bi):
                if bi >= n_obsblk or bi in obs_blocks:
                    return
                ot = obs_pool.tile([IN + 1, obs_blk * BL], b16, tag="obsblk")
                nc.sync.dma_start(
                    ot[:], obsT_d[:, bi * obs_blk * BL : (bi + 1) * obs_blk * BL]
                )
                mt = mask_pool.tile([128, obs_blk * BL], b16, tag="maskblk")
                nc.sync.dma_start(
                    mt[:], mask_d[:, bi * obs_blk * BL : (bi + 1) * obs_blk * BL]
                )
                obs_blocks[bi] = (ot, mt)

            # --- x-projection groups (psum prefill + a_n evacuation) ---
            groups = {}

            def emit_xgroup(g):
                if g >= n_grp or g in groups:
                    return
                t0 = g * G4
                bi = t0 // obs_blk
                ot, _ = obs_blocks[bi]
                off = (t0 % obs_blk) * BL
                xsl = ot[:, off : off + G4 * BL]
                prz = prz_pool.tile([128, 4, G4, BL], f32, tag="prz")
                pnx = pnx_pool.tile([128, 4, G4, BL], f32, tag="pnx")
                for mc in range(6):
                    lhs = wih_t[:, mc * 128 : (mc + 1) * 128]
                    if mc < 4:
                        out = prz[:, mc, :, :]
                        first = mc == 0
                    else:
                        out = pnx[:, mc - 2, :, :]
                        first = mc == 4
                    nc.tensor.matmul(out, lhs, xsl, start=first, stop=True,
                                     skip_group_check=True)
                an = an_pool.tile([128, 2, G4, BL], b16, tag="an")
                nc.scalar.copy(an[:], pnx[:, 2:4, :, :])
                groups[g] = (prz, pnx, an)

            # --- MLP work queue (interleaved into scan idle slots) ---
            mlp_queue = []

            def enqueue_mlp_block(kb, h1_t):
                rs = kb * NBLK * BL  # output row offset
                nrows = NBLK * BL

                def hidden_layer(w_t, b_t, in_t, out_t):
                    items = []
                    for mc in (0, 1):
                        def mk(mc=mc):
                            pm = pmlp_pool.tile([128, nrows], f32, tag="pmlp")
                            for k in (0, 1):
                                nc.tensor.matmul(
                                    pm[:],
                                    w_t[:, (k * 2 + mc) * 128 : (k * 2 + mc + 1) * 128],
                                    in_t[:, k, :],
                                    start=(k == 0),
                                    stop=(k == 1),
                                )
                            nc.scalar.activation(
                                out_t[:, mc, :], pm[:], RELU, bias=b_t[:, mc : mc + 1]
                            )
                        items.append(mk)
                    return items

                x1p = x1_pool.tile([128, 2, nrows], b16, tag="x1pp")
                x1v = x1_pool.tile([128, 2, nrows], b16, tag="x1vp")
                x2p = x2_pool.tile([128, 2, nrows], b16, tag="x2pp")
                x2v = x2_pool.tile([128, 2, nrows], b16, tag="x2vp")

                mlp_queue.extend(hidden_layer(wp1_t, bp1_t, h1_t, x1p))
                mlp_queue.extend(hidden_layer(wv1_t, bv1_t, h1_t, x1v))
                mlp_queue.extend(hidden_layer(wp2_t, bp2_t, x1p, x2p))
                mlp_queue.extend(hidden_layer(wv2_t, bv2_t, x1v, x2v))

                def l3():
                    pm3 = pout_pool.tile([33, nrows], f32, tag="pout")
                    for k in (0, 1):
                        nc.tensor.matmul(
                            pm3[0:16, :],
                            w3_t[:, k * 17 : k * 17 + 16],
                            x2p[:, k, :],
                            start=(k == 0),
                            stop=(k == 1),
                            tile_position=(0, 0),
                        )
                    for k in (0, 1):
                        nc.tensor.matmul(
                            pm3[32:33, :],
                            w3_t[:, k * 17 + 16 : k * 17 + 17],
                            x2v[:, k, :],
                            start=(k == 0),
                            stop=(k == 1),
                            tile_position=(0, 32),
                        )
                    osb = outsb_pool.tile([33, nrows], f32, tag="outsb")
                    nc.scalar.add(osb[0:16, :], pm3[0:16, :], b3_t[0:16, 0:1])
                    nc.scalar.add(osb[32:33, :], pm3[32:33, :], b3_t[32:33, 0:1])
                    nc.sync.dma_start(pol_d[:, rs : rs + nrows], osb[0:16, :])
                    nc.sync.dma_start(val_d[:, rs : rs + nrows], osb[32:33, :])

                mlp_queue.append(l3)

            # --- prologue ---
            emit_obsblock(0)
            emit_obsblock(1)
            hm = hm_pool.tile([128, 2, BL], b16, tag="hm")
            nc.sync.dma_start(hm[:], hminit_d[:])
            emit_xgroup(0)
            emit_xgroup(1)

            h1_blk = None

            # --- the scan ---
            for t in range(t_total):
                g, ti = divmod(t, G4)
                prz, pnx, an = groups[g]
                if t % obs_blk == 0:
                    emit_obsblock(t // obs_blk + 1)
                if ti == 0:
                    emit_xgroup(g + 1)
                if t % NBLK == 0:
                    h1_blk = h1_pool.tile([128, 2, NBLK * BL], b16, tag="h1")
                tb = t % NBLK

                # gate matmuls: r first (unblocks the chain), then n, bias, z
                for mc in (0, 1):  # r
                    for k in (0, 1):
                        nc.tensor.matmul(
                            prz[:, mc, ti, :],
                            whh_t[:, (k * 6 + mc) * 128 : (k * 6 + mc + 1) * 128],
                            hm[:, k, :],
                            start=False,
                            stop=(k == 1),
                            skip_group_check=True,
                        )
                for mc in (4, 5):  # n (h-part), plus b_n via ones-row matmul
                    for k in (0, 1):
                        nc.tensor.matmul(
                            pnx[:, mc - 4, ti, :],
                            whh_t[:, (k * 6 + mc) * 128 : (k * 6 + mc + 1) * 128],
                            hm[:, k, :],
                            start=False,
                            stop=False,
                            skip_group_check=True,
                        )
                    nc.tensor.matmul(
                        pnx[:, mc - 4, ti, :],
                        bn_t[:, (mc - 4) * 128 : (mc - 3) * 128],
                        ones_t[:],
                        start=False,
                        stop=True,
                        skip_group_check=True,
                    )
                # sigmoid(r) straight off PSUM
                rz = rz_pool.tile([128, 4, BL], b16, tag="rz")
                nc.scalar.activation(rz[:, 0:2, :], prz[:, 0:2, ti, :], SIG)

                for mc in (2, 3):  # z
                    for k in (0, 1):
                        nc.tensor.matmul(
                            prz[:, mc, ti, :],
                            whh_t[:, (k * 6 + mc) * 128 : (k * 6 + mc + 1) * 128],
                            hm[:, k, :],
                            start=False,
                            stop=(k == 1),
                            skip_group_check=True,
                        )

                if DBG and t == 0:
                    dtile = consts.tile([128, 2 * BL], f32, tag="dbg_an_t")
                    nc.vector.tensor_copy(dtile[:], an[:, :, 0, :])
                    nc.sync.dma_start(dbg_an_d[:], dtile[:])
                # n-gate chain: t_n = (hn + b_n) * r ; u = t_n + a_n ; n = tanh(u)
                tn = tmp_pool.tile([128, 2, BL], b16, tag="tn")
                nc.vector.tensor_mul(tn[:], pnx[:, 0:2, ti, :], rz[:, 0:2, :])
                u = tmp_pool.tile([128, 2, BL], b16, tag="u")
                nc.vector.tensor_add(u[:], tn[:], an[:, :, ti, :])
                nn_t = n_pool.tile([128, 2, BL], b16, tag="nn")
                nc.scalar.activation(nn_t[:], u[:], TANH)

                nc.scalar.activation(rz[:, 2:4, :], prz[:, 2:4, ti, :], SIG)

                if DBG and t == 0:
                    dtile2 = consts.tile([128, 4 * BL], f32, tag="dbg_rz_t")
                    nc.vector.tensor_copy(dtile2[:], rz[:])
                    nc.sync.dma_start(dbg_rz_d[:], dtile2[:])
                # blend: h1 = n + z*(hm - n); next hm = h1 * mask[t+1]
                v = tmp_pool.tile([128, 2, BL], b16, tag="v")
                nc.vector.tensor_sub(v[:], hm[:], nn_t[:])
                w = tmp_pool.tile([128, 2, BL], b16, tag="w")
                nc.vector.tensor_mul(w[:], rz[:, 2:4, :], v[:])
                h1 = h1_blk[:, :, tb * BL : (tb + 1) * BL]
                nc.vector.tensor_add(h1, nn_t[:], w[:])

                if t < t_total - 1:
                    _, mt = obs_blocks[(t + 1) // obs_blk]
                    moff = ((t + 1) % obs_blk) * BL
                    msl = (
                        mt[:, moff : moff + BL]
                        .unsqueeze(1)
                        .broadcast_to([128, 2, BL])
                    )
                    hm = hm_pool.tile([128, 2, BL], b16, tag="hm")
                    nc.vector.tensor_mul(hm[:], h1, msl)
                else:
                    hf32 = consts.tile([128, 2 * BL], f32, tag="hf32")
                    nc.vector.tensor_copy(
                        hf32[:], h1_blk[:, :, tb * BL : (tb + 1) * BL]
                    )
                    nc.sync.dma_start(hf_d[:], hf32[:])

                if t % NBLK == NBLK - 1:
                    enqueue_mlp_block(t // NBLK, h1_blk)

                budget = 2 if t % NBLK else 4
                for _ in range(min(budget, len(mlp_queue))):
                    mlp_queue.pop(0)()

            while mlp_queue:
                mlp_queue.pop(0)()

    _split_excess_waits(nc)
    return nc


def _prep_core_inputs(c, obs, done, hidden, W_ih, W_hh, b_ih, b_n, pp, vp, t_total):
    """Host-side layout prep for core c (batch slice of 32)."""
    sl = slice(c * BL, (c + 1) * BL)
    obs_c = obs[sl, :t_total]                       # [32, T, 64]
    done_c = done[sl, :t_total]                     # [32, T]
    m = 1.0 - done_c.T.astype(np.float32)           # [T, 32]
    TBc = t_total * BL

    obsT = np.empty((IN + 1, TBc), np.float32)
    obsT[:IN] = obs_c.transpose(2, 1, 0).reshape(IN, TBc)
    obsT[IN] = 1.0                                  # bias row (b_ih fold)

    maskrep = np.broadcast_to(m.reshape(1, TBc), (128, TBc))

    hT = hidden[sl].T.astype(np.float32)            # [256, 32]
    hm0 = (hT * m[0][None, :]).reshape(2, 128, BL).transpose(1, 0, 2).reshape(128, 2 * BL)

    whh = W_hh.reshape(6, 128, 2, 128).transpose(3, 2, 0, 1).reshape(128, 2 * 6 * 128)
    wih_aug = np.concatenate([W_ih, b_ih[:, None]], axis=1)  # [768, 65]
    wih = wih_aug.reshape(6, 128, IN + 1).transpose(2, 0, 1).reshape(IN + 1, 6 * 128)

    def hidw(Wmat):  # [256,256] -> [128, (k,mc,q)]
        return Wmat.reshape(2, 128, 2, 128).transpose(3, 2, 0, 1).reshape(128, 512)

    (Wp1, bp1), (Wp2, bp2), (Wp3, bp3) = pp
    (Wv1, bv1), (Wv2, bv2), (Wv3, bv3) = vp
    W3s = np.concatenate([Wp3, Wv3], axis=0)        # [17, 256]
    w3 = W3s.reshape(17, 2, 128).transpose(2, 1, 0).reshape(128, 2 * 17)
    b3 = np.zeros((33, 1), np.float32)
    b3[0:16, 0] = bp3
    b3[32, 0] = bv3[0]

    f = np.ascontiguousarray
    return {
        "obsT": f(obsT.astype(bf16)),
        "maskrep": f(maskrep.astype(bf16)),
        "hminit": f(hm0.astype(bf16)),
        "whh": f(whh.astype(bf16)),
        "wih": f(wih.astype(bf16)),
        "bn": f(b_n.reshape(1, 256).astype(bf16)),
        "wp1": f(hidw(Wp1).astype(bf16)),
        "wp2": f(hidw(Wp2).astype(bf16)),
        "wv1": f(hidw(Wv1).astype(bf16)),
        "wv2": f(hidw(Wv2).astype(bf16)),
        "w3": f(w3.astype(bf16)),
        "bp1": f(bp1.reshape(2, 128).T.astype(np.float32)),
        "bp2": f(bp2.reshape(2, 128).T.astype(np.float32)),
        "bv1": f(bv1.reshape(2, 128).T.astype(np.float32)),
        "bv2": f(bv2.reshape(2, 128).T.astype(np.float32)),
        "b3": f(b3),
    }


def run(inputs, t_total=T, trace=False):
    from concourse.bass_utils import run_bass_kernel_spmd

    obs = np.asarray(inputs["obs"], np.float32)
    done = np.asarray(inputs["done"])
    hidden = np.asarray(inputs["hidden"], np.float32)
    W_ih = np.asarray(inputs["W_ih"], np.float32)
    W_hh = np.asarray(inputs["W_hh"], np.float32)
    b_ih = np.asarray(inputs["b_ih"], np.float32)
    b_n = np.asarray(inputs["b_n"], np.float32)
    pp = [(np.asarray(W, np.float32), np.asarray(b, np.float32))
          for W, b in inputs["policy_params"]]
    vp = [(np.asarray(W, np.float32), np.asarray(b, np.float32))
          for W, b in inputs["value_params"]]

    key = t_total
    if key not in _NC_CACHE:
        _NC_CACHE[key] = build_module(t_total)
    nc = _NC_CACHE[key]

    in_maps = [
        _prep_core_inputs(c, obs, done, hidden, W_ih, W_hh, b_ih, b_n, pp, vp, t_total)
        for c in range(NCORES)
    ]
    res = run_bass_kernel_spmd(nc, in_maps, list(range(NCORES)), trace=trace)

    policy = np.empty((B, t_total, 16), np.float32)
    value = np.empty((B, t_total), np.float32)
    h_final = np.empty((B, R), np.float32)
    for c in range(NCORES):
        sl = slice(c * BL, (c + 1) * BL)
        r = res.results[c]
        policy[sl] = r["policy_out"].reshape(16, t_total, BL).transpose(2, 1, 0)
        value[sl] = r["value_out"].reshape(t_total, BL).T
        h_final[sl] = (
            r["hfinal_out"].reshape(128, 2, BL).transpose(2, 1, 0).reshape(BL, R)
        )
    return (h_final, (policy, value)), res


def kernel(**inputs):
    out, _ = run(inputs, T)
    return out
